# revision 8
# baseline (speedup 1.0000x reference)
"""Deformable-attention kernel for Trainium2 (8 NeuronCores, batch-parallel).

Problem (per sample): x (256,64,64) ->
  q = 1x1conv(x, wq)+bq
  h = gelu(conv5x5_s4(q, w1)+b1); off = 1x1(h, w2)+b2  (15x15 grid)
  dp = clip(ref+off); x_tilde = grid_sample_bilinear(x, dp)   (225 pts)
  k = wk@x_tilde+bk; v = wv@x_tilde+bv
  out = softmax(q^T k, axis=n) @ v^T   (m=4096, n=225)

Sharding: B=16 across 8 cores (2 samples/core), weights replicated.
"""
import sys

if "/opt/trn_rl_repo" not in sys.path:
    sys.path.insert(0, "/opt/trn_rl_repo")

import numpy as np

import concourse.bass as bass
import concourse.bacc as bacc
import concourse.mybir as mybir
from concourse.masks import make_identity
from concourse.tile import TileContext

P = 128
C = 256
H = W = 64
M = H * W            # 4096 per sample
NS = 2               # samples per core
MF = NS * M          # fused free dim 8192
K5, R = 5, 4
HR = WR = 15
N = HR * WR          # 225 sample points
NT = [(0, 128), (128, 97)]   # (row offset, rows) tiles of the n=225 dim
F32 = mybir.dt.float32
I32 = mybir.dt.int32
AF = mybir.ActivationFunctionType
OP = mybir.AluOpType

MSUB = 512           # matmul free-dim chunk
XCH = 1024           # x streaming chunk width


def build_nc(stages=99, finalize=True):
    nc = bacc.Bacc()

    x_d = nc.dram_tensor("x", [NS, C, M], F32, kind="ExternalInput")
    xT_d = nc.dram_tensor("xT", [NS * M, C], F32, kind="ExternalInput")
    wqT_d = nc.dram_tensor("wqT", [C, C], F32, kind="ExternalInput")
    wkT_d = nc.dram_tensor("wkT", [C, C], F32, kind="ExternalInput")
    wvT_d = nc.dram_tensor("wvT", [C, C], F32, kind="ExternalInput")
    w1t_d = nc.dram_tensor("w1t", [C, 25 * C], F32, kind="ExternalInput")
    w2T_d = nc.dram_tensor("w2T", [C, 2], F32, kind="ExternalInput")
    bq_d = nc.dram_tensor("bq", [C, 1], F32, kind="ExternalInput")
    bk_d = nc.dram_tensor("bk", [C, 1], F32, kind="ExternalInput")
    b1_d = nc.dram_tensor("b1", [C, 1], F32, kind="ExternalInput")
    bv_d = nc.dram_tensor("bv_row", [1, C], F32, kind="ExternalInput")
    ref_d = nc.dram_tensor("refb2", [N, 2], F32, kind="ExternalInput")
    out_d = nc.dram_tensor("out", [NS, C, M], F32, kind="ExternalOutput")

    with TileContext(nc) as tc:
        with tc.tile_pool(name="sb", bufs=1) as sb, \
             tc.tile_pool(name="sbx", bufs=4) as sbx, \
             tc.tile_pool(name="sbo", bufs=4) as sbo, \
             tc.tile_pool(name="sbg", bufs=2) as sbg, \
             tc.tile_pool(name="big", bufs=2) as big, \
             tc.tile_pool(name="pmm", bufs=4, space="PSUM") as pmm, \
             tc.tile_pool(name="pden", bufs=2, space="PSUM") as pden, \
             tc.tile_pool(name="paux", bufs=2, space="PSUM") as paux:

            # ---- constants / weights ----
            wqT = [sb.tile([P, C], F32, tag=f"wq{k}", name=f"wqT{k}") for k in range(2)]
            wkT = [sb.tile([P, C], F32, tag=f"wk{k}", name=f"wkT{k}") for k in range(2)]
            wvT = [sb.tile([P, C], F32, tag=f"wv{k}", name=f"wvT{k}") for k in range(2)]
            w2T = [sb.tile([P, 2], F32, tag=f"w2{k}", name=f"w2T{k}") for k in range(2)]
            for k in range(2):
                nc.sync.dma_start(out=wqT[k][:], in_=wqT_d[k * P:(k + 1) * P, :])
                nc.sync.dma_start(out=wkT[k][:], in_=wkT_d[k * P:(k + 1) * P, :])
                nc.sync.dma_start(out=wvT[k][:], in_=wvT_d[k * P:(k + 1) * P, :])
                nc.sync.dma_start(out=w2T[k][:], in_=w2T_d[k * P:(k + 1) * P, :])
            bq = sb.tile([P, 2], F32, tag="bq")
            bk = sb.tile([P, 2], F32, tag="bk")
            b1 = sb.tile([P, 2], F32, tag="b1")
            for k in range(2):
                nc.sync.dma_start(out=bq[:, k:k + 1], in_=bq_d[k * P:(k + 1) * P, :])
                nc.sync.dma_start(out=bk[:, k:k + 1], in_=bk_d[k * P:(k + 1) * P, :])
                nc.sync.dma_start(out=b1[:, k:k + 1], in_=b1_d[k * P:(k + 1) * P, :])
            bv_row = sb.tile([1, C], F32, tag="bv")
            nc.sync.dma_start(out=bv_row[:], in_=bv_d[:, :])
            refb2 = [sb.tile([P, 2], F32, tag=f"ref{k}", name=f"refb2{k}") for k in range(2)]
            for t, (o, r) in enumerate(NT):
                nc.sync.dma_start(out=refb2[t][:r, :], in_=ref_d[o:o + r, :])
            ones = sb.tile([P, P], F32, tag="ones")
            nc.vector.memset(ones[:], 1.0)
            ident = sb.tile([P, P], F32, tag="ident")
            make_identity(nc, ident[:])

            # conv weights (prefetch; released before expS reuses the slots)
            w1t = [big.tile([P, 25 * C], F32, tag="big", name=f"w1t{k}") for k in range(2)]
            for k in range(2):
                nc.sync.dma_start(out=w1t[k][:], in_=w1t_d[k * P:(k + 1) * P, :])

            # ---- stage B: q = wqT.T @ x + bq, streaming x in chunks ----
            q = [sb.tile([P, MF], F32, tag=f"q{ct}", name=f"q{ct}") for ct in range(2)]
            for ch in range(MF // XCH):
                xch = [sbx.tile([P, XCH], F32, tag="xst", name=f"xch{ch}_{k}")
                       for k in range(2)]
                for k in range(2):
                    s, mo = divmod(ch * XCH, M)
                    nc.sync.dma_start(out=xch[k][:],
                                      in_=x_d[s, k * P:(k + 1) * P, mo:mo + XCH])
                for ct in range(2):
                    for sub in range(XCH // MSUB):
                        qp = pmm.tile([P, MSUB], F32, tag="mm", space="PSUM")
                        for k in range(2):
                            nc.tensor.matmul(
                                out=qp[:], lhsT=wqT[k][:, ct * P:(ct + 1) * P],
                                rhs=xch[k][:, sub * MSUB:(sub + 1) * MSUB],
                                start=(k == 0), stop=(k == 1))
                        nc.vector.tensor_scalar(
                            out=q[ct][:, ch * XCH + sub * MSUB:
                                      ch * XCH + (sub + 1) * MSUB],
                            in0=qp[:], scalar1=bq[:, ct:ct + 1], scalar2=None,
                            op0=OP.add)

            if stages < 2:
                raise SystemExit('stage gating unsupported after finalize change')
            # ---- stage C: conv5x5 stride4 + gelu -> h_gelu [o, 450] ----
            h_gelu = [sb.tile([P, NS * N], F32, tag=f"hg{ct}", name=f"hg{ct}") for ct in range(2)]
            for ct in range(2):
                hp = pmm.tile([P, NS * N], F32, tag="mm", space="PSUM")
                i = 0
                for di in range(K5):
                    for dj in range(K5):
                        for k in range(2):
                            qv = q[k][:].rearrange("p (s h w) -> p s h w",
                                                   s=NS, h=H, w=W)
                            rhs = qv[:, :, di:di + 4 * (HR - 1) + 1:R,
                                     dj:dj + 4 * (WR - 1) + 1:R]
                            nc.tensor.matmul(
                                out=hp[:],
                                lhsT=w1t[k][:, (di * K5 + dj) * C + ct * P:
                                            (di * K5 + dj) * C + (ct + 1) * P],
                                rhs=rhs, start=(i == 0), stop=(i == 49))
                            i += 1
                nc.scalar.activation(out=h_gelu[ct][:], in_=hp[:], func=AF.Gelu,
                                     bias=b1[:, ct:ct + 1], scale=1.0)

            if stages < 3:
                return nc
            # ---- stage D: offsets -> sample positions -> gather x_tilde ----
            # xTg[s][t]: gathered+blended [rows, C] (rows of x^T at bilinear taps)
            xTg = [[sbg.tile([P, C], F32, tag="xTg", bufs=4, name=f"xTg{s}{t}")
                    for t in range(len(NT))] for s in range(NS)]
            for s in range(NS):
                for t, (o, r) in enumerate(NT):
                    op_ = paux.tile([P, 2], F32, tag="aux", space="PSUM")
                    for k in range(2):
                        nc.tensor.matmul(
                            out=op_[:r, :],
                            lhsT=h_gelu[k][:, s * N + o:s * N + o + r],
                            rhs=w2T[k][:], start=(k == 0), stop=(k == 1))
                    # dp = clip(refb2 + off, -1, 1); g = 31.5*dp + 31.5 in [0,63]
                    dg = sbo.tile([P, 2], F32, tag="dg")
                    nc.vector.tensor_tensor(out=dg[:r, :], in0=op_[:r, :],
                                            in1=refb2[t][:r, :], op=OP.add)
                    nc.vector.tensor_scalar(out=dg[:r, :], in0=dg[:r, :],
                                            scalar1=1.0, scalar2=-1.0,
                                            op0=OP.min, op1=OP.max)
                    nc.vector.tensor_scalar(out=dg[:r, :], in0=dg[:r, :],
                                            scalar1=31.5, scalar2=31.5,
                                            op0=OP.mult, op1=OP.add)
                    # f0 = floor(g) (trunc cast; robust fixup if conversion rounds up)
                    i0 = sbo.tile([P, 2], I32, tag="i0")
                    f0 = sbo.tile([P, 2], F32, tag="f0")
                    fx = sbo.tile([P, 2], F32, tag="fx")
                    nc.vector.tensor_copy(out=i0[:r, :], in_=dg[:r, :])
                    nc.vector.tensor_copy(out=f0[:r, :], in_=i0[:r, :])
                    nc.vector.tensor_tensor(out=fx[:r, :], in0=f0[:r, :],
                                            in1=dg[:r, :], op=OP.is_gt)
                    nc.vector.tensor_tensor(out=f0[:r, :], in0=f0[:r, :],
                                            in1=fx[:r, :], op=OP.subtract)
                    # w1c = frac, w0c = 1-frac; f1 = min(f0+1, 63)
                    w1c = sbo.tile([P, 2], F32, tag="w1c")
                    w0c = sbo.tile([P, 2], F32, tag="w0c")
                    f1 = sbo.tile([P, 2], F32, tag="f1")
                    nc.vector.tensor_tensor(out=w1c[:r, :], in0=dg[:r, :],
                                            in1=f0[:r, :], op=OP.subtract)
                    nc.vector.tensor_scalar(out=w0c[:r, :], in0=w1c[:r, :],
                                            scalar1=-1.0, scalar2=1.0,
                                            op0=OP.mult, op1=OP.add)
                    nc.vector.tensor_scalar(out=f1[:r, :], in0=f0[:r, :],
                                            scalar1=1.0, scalar2=63.0,
                                            op0=OP.add, op1=OP.min)
                    # per-tap flat indices  idx = y*64 + x  (exact in fp32)
                    ya = sbo.tile([P, 2], F32, tag="ya")   # col0: f0y*64, col1: f1y*64
                    nc.vector.tensor_scalar(out=ya[:r, 0:1], in0=f0[:r, 0:1],
                                            scalar1=64.0, scalar2=float(s * M),
                                            op0=OP.mult, op1=OP.add)
                    nc.vector.tensor_scalar(out=ya[:r, 1:2], in0=f1[:r, 0:1],
                                            scalar1=64.0, scalar2=float(s * M),
                                            op0=OP.mult, op1=OP.add)
                    acc = xTg[s][t]
                    first = True
                    for (yi, xi) in ((0, 0), (0, 1), (1, 0), (1, 1)):
                        idxf = sbo.tile([P, 1], F32, tag="idxf")
                        idxi = sbo.tile([P, 1], I32, tag="idxi")
                        xsrc = f0 if xi == 0 else f1
                        nc.vector.tensor_tensor(out=idxf[:r, :],
                                                in0=ya[:r, yi:yi + 1],
                                                in1=xsrc[:r, 1:2], op=OP.add)
                        nc.vector.tensor_copy(out=idxi[:r, :], in_=idxf[:r, :])
                        wgt = sbo.tile([P, 1], F32, tag="wgt")
                        nc.vector.tensor_tensor(
                            out=wgt[:r, :],
                            in0=(w0c if yi == 0 else w1c)[:r, 0:1],
                            in1=(w0c if xi == 0 else w1c)[:r, 1:2], op=OP.mult)
                        gt_ = sbg.tile([P, C], F32, tag="gt")
                        nc.gpsimd.indirect_dma_start(
                            out=gt_[:r, :], out_offset=None, in_=xT_d[:, :],
                            in_offset=bass.IndirectOffsetOnAxis(ap=idxi[:r, :1],
                                                                axis=0))
                        if first:
                            nc.vector.tensor_scalar(out=acc[:r, :], in0=gt_[:r, :],
                                                    scalar1=wgt[:r, :1],
                                                    scalar2=None, op0=OP.mult)
                            first = False
                        else:
                            nc.vector.scalar_tensor_tensor(
                                out=acc[:r, :], in0=gt_[:r, :], scalar=wgt[:r, :1],
                                in1=acc[:r, :], op0=OP.mult, op1=OP.add)

            if stages < 4:
                return nc
            # ---- stage E: transpose xTg -> xt [c, 450] ----
            xt = [sb.tile([P, NS * N], F32, tag=f"xt{ct}", name=f"xt{ct}") for ct in range(2)]
            for s in range(NS):
                for t, (o, r) in enumerate(NT):
                    for ct in range(2):
                        tp = paux.tile([P, P], F32, tag="aux", space="PSUM")
                        nc.tensor.transpose(out=tp[:, :r],
                                            in_=xTg[s][t][:r, ct * P:(ct + 1) * P],
                                            identity=ident[:r, :r])
                        nc.vector.tensor_copy(
                            out=xt[ct][:, s * N + o:s * N + o + r], in_=tp[:, :r])

            if stages < 5:
                return nc
            # ---- stage F: kf = wkT.T@xt + bk ; vfT = xt.T@wvT + bv ----
            kf = [sb.tile([P, NS * N], F32, tag=f"kf{ct}", name=f"kf{ct}") for ct in range(2)]
            for ct in range(2):
                kp = pmm.tile([P, NS * N], F32, tag="mm", space="PSUM")
                for k in range(2):
                    nc.tensor.matmul(out=kp[:], lhsT=wkT[k][:, ct * P:(ct + 1) * P],
                                     rhs=xt[k][:], start=(k == 0), stop=(k == 1))
                nc.scalar.activation(out=kf[ct][:], in_=kp[:], func=AF.Identity,
                                     bias=bk[:, ct:ct + 1], scale=1.0)
            vfT = [[sbg.tile([P, C], F32, tag="vfT", bufs=4, name=f"vfT{s}{t}")
                   for t in range(len(NT))] for s in range(NS)]
            for s in range(NS):
                for t, (o, r) in enumerate(NT):
                    vp = paux.tile([P, C], F32, tag="aux", space="PSUM")
                    for k in range(2):
                        nc.tensor.matmul(out=vp[:r, :],
                                         lhsT=xt[k][:, s * N + o:s * N + o + r],
                                         rhs=wvT[k][:], start=(k == 0), stop=False)
                    nc.tensor.matmul(out=vp[:r, :], lhsT=ones[:1, :r],
                                     rhs=bv_row[:1, :], start=False, stop=True)
                    nc.vector.tensor_copy(out=vfT[s][t][:r, :], in_=vp[:r, :])

            if stages < 6:
                return nc
            # ---- stage G: scoresT = kf.T@q -> exp ----
            expS = [big.tile([P, MF], F32, tag="big", name=f"expS{k}") for k in range(2)]
            for s in range(NS):
                for t, (o, r) in enumerate(NT):
                    for sub in range(M // MSUB):
                        sp = pmm.tile([P, MSUB], F32, tag="mm", space="PSUM")
                        for k in range(2):
                            nc.tensor.matmul(
                                out=sp[:r, :],
                                lhsT=kf[k][:, s * N + o:s * N + o + r],
                                rhs=q[k][:, s * M + sub * MSUB:
                                         s * M + (sub + 1) * MSUB],
                                start=(k == 0), stop=(k == 1))
                        nc.scalar.activation(
                            out=expS[t][:r, s * M + sub * MSUB:
                                        s * M + (sub + 1) * MSUB],
                            in_=sp[:r, :], func=AF.Exp)

            if stages < 7:
                return nc
            # ---- stage H: den + att@v + normalize -> out ----
            for s in range(NS):
                for sub in range(M // MSUB):
                    msl = slice(s * M + sub * MSUB, s * M + (sub + 1) * MSUB)
                    dp_ = pden.tile([P, MSUB], F32, tag="den", space="PSUM")
                    for t, (o, r) in enumerate(NT):
                        nc.tensor.matmul(out=dp_[:], lhsT=ones[:r, :],
                                         rhs=expS[t][:r, msl],
                                         start=(t == 0), stop=(t == 1))
                    invd = sbx.tile([P, MSUB], F32, tag="invd")
                    nc.vector.reciprocal(out=invd[:], in_=dp_[:])
                    for ct in range(2):
                        ap_ = pmm.tile([P, MSUB], F32, tag="mm", space="PSUM")
                        for t, (o, r) in enumerate(NT):
                            nc.tensor.matmul(
                                out=ap_[:],
                                lhsT=vfT[s][t][:r, ct * P:(ct + 1) * P],
                                rhs=expS[t][:r, msl],
                                start=(t == 0), stop=(t == 1))
                        ost = sbo.tile([P, MSUB], F32, tag="ost")
                        nc.vector.tensor_tensor(out=ost[:], in0=ap_[:],
                                                in1=invd[:], op=OP.mult)
                        nc.sync.dma_start(
                            out=out_d[s, ct * P:(ct + 1) * P,
                                      sub * MSUB:(sub + 1) * MSUB],
                            in_=ost[:])
    if finalize:
        nc.finalize()
    return nc


def _prep_shared(w1, w2, b2, wq, wk, wv, bq, bk, bv, b1):
    ry = (np.linspace(0.5, HR - 0.5, HR, dtype=np.float32) / (HR - 1.0)) * 2.0 - 1.0
    rx = (np.linspace(0.5, WR - 0.5, WR, dtype=np.float32) / (WR - 1.0)) * 2.0 - 1.0
    ref = np.stack(np.meshgrid(ry, rx, indexing="ij"), axis=-1).reshape(N, 2)
    return {
        "wqT": np.ascontiguousarray(wq.T),
        "wkT": np.ascontiguousarray(wk.T),
        "wvT": np.ascontiguousarray(wv.T),
        "w1t": np.ascontiguousarray(w1.transpose(1, 2, 3, 0).reshape(C, 25 * C)),
        "w2T": np.ascontiguousarray(w2.T),
        "bq": bq.reshape(C, 1).copy(),
        "bk": bk.reshape(C, 1).copy(),
        "b1": b1.reshape(C, 1).copy(),
        "bv_row": bv.reshape(1, C).copy(),
        "refb2": (ref + b2.reshape(1, 2)).astype(np.float32),
    }


_NC_CACHE = {}


def _get_nc():
    if "nc" not in _NC_CACHE:
        _NC_CACHE["nc"] = build_nc()
    return _NC_CACHE["nc"]


def make_in_maps(x, w1, b1, w2, b2, wq, bq, wk, bk, wv, bv):
    shared = _prep_shared(w1, w2, b2, wq, wk, wv, bq, bk, bv, b1)
    B = x.shape[0]
    n_cores = 8
    per = B // n_cores
    in_maps = []
    for c in range(n_cores):
        xs = np.ascontiguousarray(
            x[c * per:(c + 1) * per].reshape(per, C, M).astype(np.float32))
        m = dict(shared)
        m["x"] = xs
        m["xT"] = np.ascontiguousarray(xs.transpose(0, 2, 1)).reshape(NS * M, C)
        in_maps.append(m)
    return in_maps


def kernel(x, w1, b1, w2, b2, wq, bq, wk, bk, wv, bv):
    from concourse.bass_utils import run_bass_kernel_spmd

    nc = _get_nc()
    in_maps = make_in_maps(x, w1, b1, w2, b2, wq, bq, wk, bk, wv, bv)
    res = run_bass_kernel_spmd(nc, in_maps, core_ids=list(range(8)))
    outs = [res.results[i]["out"].reshape(NS, C, H, W) for i in range(8)]
    return np.concatenate(outs, axis=0)
